# revision 14
# baseline (speedup 1.0000x reference)
"""Causal self-attention (B=2, T=2048, C=1024, H=16) on 8 TRN2 NeuronCores.

Sharding: tensor-parallel over heads — core c owns heads {2c, 2c+1} for both
batches (qkv_w column slice, o_w row slice). Each core computes a partial
o_proj output; the host sums the 8 partials and adds o_b.

Kernel math (per core), all matmuls in float32r (TF32-like, full PE rate):
  xT = transpose(x_b)                      (PE transpose-mode, 128x128 blocks)
  qT/kT/vT = W_slice^T @ x^T + bias        (weights stationary, N=512 moving)
  per (batch, head):  S^T[j,i] = kT^T qT   (K=64; heads packed at partition 0/64)
  P^T = exp(S^T/8)                         (ACT, PSUM->SBUF, [128,1024] groups)
  causal: lower j-blocks skipped, diagonal blocks masked by a 128x128 tri mask
  O_aug^T[d+1, i] = V_aug^T P^T            (V augmented with a ones column ->
                                            row 64 of the PSUM accumulator is
                                            the softmax denominator, free)
  attT = O^T * (1/denominator)             (DVE, denom row broadcast via DMA)
  Y[t, e] = attT^T @ o_w_slice             (attT stationary, N=512 moving)
"""

import numpy as np

B = 2
T = 2048
C = 1024
H = 16
DH = 64
NCORES = 8
HL = 2                      # heads per core
HCOLS = HL * DH             # 128
TB = T // 128               # 16 t-blocks per batch
KB = C // 128               # 8 k-blocks
NCH = T // 512              # 4 i-chunks per batch

_nc_cache = None


def build_bass(dbg=False):
    import concourse.bass as bass
    import concourse.bacc as bacc
    import concourse.tile as tile
    import concourse.mybir as mybir

    F32 = mybir.dt.float32
    F32R = mybir.dt.float32r

    def r(ap):
        return ap

    nc = bacc.Bacc("TRN2", target_bir_lowering=False, debug=False)

    x_d = nc.dram_tensor("x", [B, T, C], F32R, kind="ExternalInput")
    w_d = nc.dram_tensor("w", [C, 3 * HCOLS], F32R, kind="ExternalInput")
    bias_d = nc.dram_tensor("bqkv", [HCOLS, 3], F32, kind="ExternalInput")
    ow_d = nc.dram_tensor("ow", [HCOLS, C], F32R, kind="ExternalInput")
    ident_d = nc.dram_tensor("ident", [128, 128], F32R, kind="ExternalInput")
    tri_d = nc.dram_tensor("tri", [128, 128], F32R, kind="ExternalInput")
    ones_d = nc.dram_tensor("ones", [128, TB], F32R, kind="ExternalInput")
    y_d = nc.dram_tensor("y", [B * T, C], F32, kind="ExternalOutput")
    if dbg:
        qT_dbg = nc.dram_tensor("qT_dbg", [128, B * T], F32, kind="ExternalOutput")
        kT_dbg = nc.dram_tensor("kT_dbg", [128, B * T], F32, kind="ExternalOutput")
        vT_dbg = nc.dram_tensor("vT_dbg", [B, 128, T], F32, kind="ExternalOutput")
        attT_dbg = nc.dram_tensor("attT_dbg", [128, B * T], F32, kind="ExternalOutput")
        xT_dbg = nc.dram_tensor("xT_dbg", [128, KB * (T // 2)], F32, kind="ExternalOutput")

    with tile.TileContext(nc) as tc:
        with (
            tc.tile_pool(name="const", bufs=1) as constp,
            tc.tile_pool(name="xnat", bufs=2) as xnatp,
            tc.tile_pool(name="xT", bufs=1) as xtp,
            tc.tile_pool(name="qkv", bufs=1) as qkvp,
            tc.tile_pool(name="vaug", bufs=2) as vaugp,
            tc.tile_pool(name="pT", bufs=4) as ptp,
            tc.tile_pool(name="att", bufs=1) as attp,
            tc.tile_pool(name="recip", bufs=2) as recipp,
            tc.tile_pool(name="yout", bufs=2) as youtp,
            tc.tile_pool(name="ps", bufs=1, space="PSUM") as ps,
        ):
            # ---- constants / weights ----
            w_sb = constp.tile([128, KB * 3 * 128], F32R)      # [k, (kb, m*128)]
            ow_sb = constp.tile([128, C], F32R)
            bias_sb = constp.tile([HCOLS, 3], F32)
            ident_sb = constp.tile([128, 128], F32R)
            tri_sb = constp.tile([128, 128], F32R)
            nc.sync.dma_start(
                w_sb[:].rearrange("p (kb d) -> p kb d", d=3 * 128),
                w_d.rearrange("(kb p) d -> p kb d", p=128),
            )
            nc.sync.dma_start(ow_sb[:], ow_d[:])
            nc.sync.dma_start(bias_sb[:], bias_d[:])
            nc.sync.dma_start(ident_sb[:], ident_d[:])
            nc.sync.dma_start(tri_sb[:], tri_d[:])

            xT = xtp.tile([128, KB * (T // 2)], F32R)          # [k, (kb, t)] half batch
            qT = qkvp.tile([128, B * T], F32R, name="qT")      # [2 heads*64, (b, t)]
            kT = qkvp.tile([128, B * T], F32R, name="kT")
            vT = qkvp.tile([128, T], F32R, name="vT")
            attT = attp.tile([128, B * T], F32R)               # [2 heads*64, (b, t)]

            def xpose_qkv(b):
                """Build xT for batch b (half at a time), then qT/kT/vT."""
                TH = T // 2
                w3 = w_sb[:].rearrange("p (kb d) -> p kb d", d=3 * 128)
                for half in range(2):
                    for tbl in range(TB // 2):
                        tb = (TB // 2) * half + tbl
                        xnat = xnatp.tile([128, C], F32R, name="xnat")
                        nc.sync.dma_start(xnat[:], x_d[b, tb * 128:(tb + 1) * 128, :])
                        for g in range(2):  # groups of 4 k-blocks -> one bank
                            pt = ps.tile([128, 512], F32R, name="psx", tag="b1", bufs=4)
                            for i in range(4):
                                kb = 4 * g + i
                                nc.tensor.transpose(
                                    pt[:, i * 128:(i + 1) * 128],
                                    xnat[:, kb * 128:(kb + 1) * 128],
                                    ident_sb[:],
                                )
                            dst = xT[:].rearrange("p (kb t) -> p kb t", t=TH)[
                                :, 4 * g:4 * g + 4, tbl * 128:(tbl + 1) * 128]
                            src = pt[:].rearrange("p (i t) -> p i t", t=128)
                            if (tb + g) % 2 == 0:
                                nc.vector.tensor_copy(dst, src)
                            else:
                                nc.scalar.activation(
                                    dst, src, mybir.ActivationFunctionType.Copy)
                    # projection for this half: out = qkv^T [dcol, t-chunk]
                    for m, dstT in ((0, qT), (1, kT), (2, vT)):
                        for tcl in range(NCH // 2):
                            tc_ = (NCH // 2) * half + tcl
                            pt = ps.tile([128, 512], F32, name="psqkv", tag="b1",
                                         bufs=4)
                            for kb in range(KB):
                                nc.tensor.matmul(
                                    pt[:],
                                    r(w3[:, kb, m * 128:(m + 1) * 128]),
                                    r(xT[:, kb * TH + tcl * 512:
                                          kb * TH + tcl * 512 + 512]),
                                    start=(kb == 0),
                                    stop=(kb == KB - 1),
                                )
                            vdst = dstT[:, tc_ * 512: tc_ * 512 + 512] if dstT is vT \
                                else dstT[:, b * T + tc_ * 512: b * T + tc_ * 512 + 512]
                            nc.vector.tensor_scalar_add(
                                vdst, pt[:], bias_sb[:, m:m + 1])

            def build_vaug(b):
                """v_aug[j, 16 jb-blocks x (64 v cols + ones col)] per head."""
                vaugs = []
                for h in range(HL):
                    va = vaugp.tile([128, TB * 65], F32R, name=f"vaug{h}", tag=f"va{h}")
                    nc.sync.dma_start(
                        va[:].rearrange("p (tb d) -> p tb d", d=65)[:, :, 64:65],
                        ones_d.rearrange("p (tb o) -> p tb o", o=1))
                    for g in range(2):  # 8 transposes -> one PSUM bank
                        pt = ps.tile([128, 512], F32R, name="psva", tag="b1", bufs=4)
                        for i in range(8):
                            tb = 8 * g + i
                            nc.tensor.transpose(
                                pt[:, i * 64:(i + 1) * 64],
                                vT[h * 64:(h + 1) * 64,
                                   tb * 128:(tb + 1) * 128],
                                ident_sb[h * 64:(h + 1) * 64, h * 64:(h + 1) * 64],
                            )
                        dst = va[:].rearrange("p (tb d) -> p tb d", d=65)[
                            :, 8 * g:8 * g + 8, 0:64]
                        src = pt[:].rearrange("p (i d) -> p i d", d=64)
                        nc.vector.tensor_copy(dst, src)
                    vaugs.append(va)
                return vaugs

            def attention(b, vaugs):
                """Attention for both heads of batch b (heads packed on PE)."""
                for ic in range(NCH):
                    i0 = 512 * ic
                    n_jb = 4 * (ic + 1)
                    pv = [ps.tile([128, 512], F32, name=f"pspv{h}", tag="b1", bufs=4)
                          for h in range(HL)]
                    for g in range(n_jb // 2):
                        sc = [ps.tile([128, 1024], F32, name=f"pssc{h}", tag="b2", bufs=2)
                              for h in range(HL)]
                        # scores: S^T[j, i] = kT_blk^T @ qT_chunk  (K=64 each head)
                        for u in range(2):
                            jb = 2 * g + u
                            j0 = 128 * jb
                            for h in range(HL):
                                nc.tensor.matmul(
                                    sc[h][:, u * 512:(u + 1) * 512],
                                    r(kT[h * 64:(h + 1) * 64,
                                         b * T + j0: b * T + j0 + 128]),
                                    r(qT[h * 64:(h + 1) * 64,
                                         b * T + i0: b * T + i0 + 512]),
                                    start=True, stop=True,
                                )
                        pts = []
                        for h in range(HL):
                            pt_sb = ptp.tile([128, 1024], F32R, name=f"pt{h}",
                                             tag=f"pt{h}", bufs=2)
                            nc.scalar.activation(
                                pt_sb[:], sc[h][:],
                                mybir.ActivationFunctionType.Exp,
                                scale=float(1.0 / np.sqrt(DH)),
                            )
                            pts.append(pt_sb)
                        for u in range(2):
                            jb = 2 * g + u
                            o = 128 * jb - i0
                            lo = max(o, 0)
                            for h in range(HL):
                                if o >= 0:  # diagonal block: triangular mask
                                    seg = pts[h][:, u * 512 + o:
                                                 u * 512 + o + 128]
                                    nc.vector.tensor_tensor(
                                        seg, seg, tri_sb[:], mybir.AluOpType.mult)
                                nc.tensor.matmul(
                                    pv[h][0:65, lo:512],
                                    r(vaugs[h][:, jb * 65: jb * 65 + 65]),
                                    r(pts[h][:, u * 512 + lo:(u + 1) * 512]),
                                    start=(jb == 0),
                                    stop=(jb == n_jb - 1),
                                )
                    # normalize + evacuate attT
                    for h in range(HL):
                        rrow = recipp.tile([1, 512], F32, name="rrow", tag="rr")
                        rbc = recipp.tile([64, 512], F32, name="rbc", tag="rb")
                        nc.vector.reciprocal(rrow[:], pv[h][64:65, :])
                        nc.gpsimd.partition_broadcast(rbc[:], rrow[:])
                        nc.vector.tensor_tensor(
                            attT[h * 64:(h + 1) * 64,
                                 b * T + i0: b * T + i0 + 512],
                            pv[h][0:64, :], rbc[:], mybir.AluOpType.mult)

            def oproj(b):
                for tb in range(TB):
                    yo = youtp.tile([128, C], F32, name="yo")
                    for ec in range(2):
                        pt = ps.tile([128, 512], F32, name="psy", tag="b1", bufs=4)
                        nc.tensor.matmul(
                            pt[:],
                            r(attT[:, b * T + tb * 128: b * T + (tb + 1) * 128]),
                            r(ow_sb[:, ec * 512:(ec + 1) * 512]),
                            start=True, stop=True,
                        )
                        dst = yo[:, ec * 512:(ec + 1) * 512]
                        if ec == 0:
                            nc.vector.tensor_copy(dst, pt[:])
                        else:
                            nc.scalar.activation(
                                dst, pt[:], mybir.ActivationFunctionType.Copy)
                    nc.sync.dma_start(
                        y_d[b * T + tb * 128: b * T + (tb + 1) * 128, :], yo[:])

            # ---- schedule ----
            xpose_qkv(0)
            if dbg:
                nc.sync.dma_start(xT_dbg[:], xT[:].bitcast(F32))
                nc.sync.dma_start(vT_dbg[0], vT[:].bitcast(F32))
            va0 = build_vaug(0)
            attention(0, va0)
            xpose_qkv(1)
            if dbg:
                nc.sync.dma_start(vT_dbg[1], vT[:].bitcast(F32))
                nc.sync.dma_start(qT_dbg[:], qT[:].bitcast(F32))
                nc.sync.dma_start(kT_dbg[:], kT[:].bitcast(F32))
            oproj(0)
            va1 = build_vaug(1)
            attention(1, va1)
            oproj(1)
            if dbg:
                nc.sync.dma_start(attT_dbg[:], attT[:].bitcast(F32))

    nc.compile()
    return nc


def _prep_inputs(x, qkv_w, qkv_b, o_w):
    """Per-core input maps (head sharding)."""
    ident = np.eye(128, dtype=np.float32)
    tri = np.triu(np.ones((128, 128), dtype=np.float32))
    x = np.ascontiguousarray(np.asarray(x, dtype=np.float32))
    qkv_w = np.asarray(qkv_w, dtype=np.float32)
    qkv_b = np.asarray(qkv_b, dtype=np.float32)
    o_w = np.asarray(o_w, dtype=np.float32)
    in_maps = []
    for c in range(NCORES):
        lo = c * HCOLS
        w_c = np.concatenate(
            [qkv_w[:, lo:lo + HCOLS],
             qkv_w[:, C + lo:C + lo + HCOLS],
             qkv_w[:, 2 * C + lo:2 * C + lo + HCOLS]], axis=1)
        b_c = np.stack(
            [qkv_b[lo:lo + HCOLS],
             qkv_b[C + lo:C + lo + HCOLS],
             qkv_b[2 * C + lo:2 * C + lo + HCOLS]], axis=1)
        ow_c = o_w[lo:lo + HCOLS, :]
        in_maps.append({
            "x": x,
            "w": np.ascontiguousarray(w_c),
            "bqkv": np.ascontiguousarray(b_c),
            "ow": np.ascontiguousarray(ow_c),
            "ident": ident,
            "tri": tri,
            "ones": np.ones((128, TB), dtype=np.float32),
        })
    return in_maps


def kernel(x, qkv_w, qkv_b, o_w, o_b):
    global _nc_cache
    from concourse import bass_utils
    if _nc_cache is None:
        _nc_cache = build_bass()
    nc = _nc_cache
    in_maps = _prep_inputs(x, qkv_w, qkv_b, o_w)
    res = bass_utils.run_bass_kernel_spmd(nc, in_maps, core_ids=list(range(NCORES)))
    y = np.zeros((B * T, C), dtype=np.float64)
    for c in range(NCORES):
        y += res.results[c]["y"].astype(np.float64)
    y = (y + np.asarray(o_b, dtype=np.float64)[None, :]).astype(np.float32)
    return y.reshape(B, T, C)


# revision 19
# speedup vs baseline: 1.1858x; 1.1858x over previous
"""Causal self-attention (B=2, T=2048, C=1024, H=16) on 8 TRN2 NeuronCores.

Sharding: tensor-parallel over heads — core c owns heads {2c, 2c+1} for both
batches (qkv_w column slice, o_w row slice). Each core computes a partial
o_proj output; the host sums the 8 partials and adds o_b.

Kernel math (per core), all matmuls in float32r (TF32-like, full PE rate):
  xT = transpose(x_b)                      (PE transpose-mode, 128x128 blocks)
  qT/kT/vT = W_slice^T @ x^T + bias        (weights stationary, N=512 moving)
  per (batch, head):  S^T[j,i] = kT^T qT   (K=64; heads packed at partition 0/64)
  P^T = exp(S^T/8)                         (ACT, PSUM->SBUF, [128,1024] groups)
  causal: lower j-blocks skipped, diagonal blocks masked by a 128x128 tri mask
  O_aug^T[d+1, i] = V_aug^T P^T            (V augmented with a ones column ->
                                            row 64 of the accumulator is the
                                            softmax denominator, zero cost)
  attT = O^T * (1/denominator)             (DVE; denom broadcast via GpSimd)
  Y[t, e] = attT^T @ o_w_slice             (attT stationary, N=512 moving)

Emission is unit-interleaved so PE stays dense while ACT runs the exps:
  [xpose+qkv b0] [attn b0 || xpose+qkv b1] [attn b1 || oproj b0] [oproj b1]
"""

import numpy as np

B = 2
T = 2048
C = 1024
H = 16
DH = 64
NCORES = 8
HL = 2                      # heads per core
HCOLS = HL * DH             # 128
TB = T // 128               # 16 t-blocks per batch
KB = C // 128               # 8 k-blocks
NCH = T // 512              # 4 i-chunks per batch
TH = T // 2

CFG = {"b1": 2, "b2": 2, "pv": 2, "pt": 4}

_nc_cache = None


def _interleave(primary, filler):
    """Emit primary units with filler units woven in (filler spread evenly)."""
    np_, nf = len(primary), len(filler)
    fi = 0
    for i, u in enumerate(primary):
        u()
        want = int(round((i + 1) * nf / max(np_, 1)))
        while fi < want:
            filler[fi]()
            fi += 1
    while fi < nf:
        filler[fi]()
        fi += 1


def build_bass(dbg=False):
    import concourse.bass as bass
    import concourse.bacc as bacc
    import concourse.tile as tile
    import concourse.mybir as mybir

    F32 = mybir.dt.float32
    F32R = mybir.dt.float32r
    Exp = mybir.ActivationFunctionType.Exp

    nc = bacc.Bacc("TRN2", target_bir_lowering=False, debug=False)

    x_d = nc.dram_tensor("x", [B, T, C], F32R, kind="ExternalInput")
    w_d = nc.dram_tensor("w", [C, 3 * HCOLS], F32R, kind="ExternalInput")
    bias_d = nc.dram_tensor("bqkv", [HCOLS, 3], F32, kind="ExternalInput")
    ow_d = nc.dram_tensor("ow", [HCOLS, C], F32R, kind="ExternalInput")
    ident_d = nc.dram_tensor("ident", [128, 128], F32R, kind="ExternalInput")
    tri_d = nc.dram_tensor("tri", [128, 128], F32R, kind="ExternalInput")
    ones_d = nc.dram_tensor("ones", [128, TB], F32R, kind="ExternalInput")
    y_d = nc.dram_tensor("y", [B * T, C], F32, kind="ExternalOutput")
    if dbg:
        qT_dbg = nc.dram_tensor("qT_dbg", [128, B * T], F32, kind="ExternalOutput")
        kT_dbg = nc.dram_tensor("kT_dbg", [128, B * T], F32, kind="ExternalOutput")
        attT_dbg = nc.dram_tensor("attT_dbg", [128, B * T], F32, kind="ExternalOutput")

    with tile.TileContext(nc) as tc:
        with (
            tc.tile_pool(name="const", bufs=1) as constp,
            tc.tile_pool(name="xnat", bufs=4) as xnatp,
            tc.tile_pool(name="xT", bufs=1) as xtp,
            tc.tile_pool(name="qkv", bufs=1) as qkvp,
            tc.tile_pool(name="vaug", bufs=2) as vaugp,
            tc.tile_pool(name="pT", bufs=4) as ptp,
            tc.tile_pool(name="att", bufs=1) as attp,
            tc.tile_pool(name="recip", bufs=2) as recipp,
            tc.tile_pool(name="yout", bufs=3) as youtp,
            tc.tile_pool(name="ps", bufs=1, space="PSUM") as ps,
        ):
            # ---- constants / weights ----
            w_sb = constp.tile([128, KB * 3 * 128], F32R)      # [k, (kb, m*128)]
            ow_sb = constp.tile([128, C], F32R)
            bias_sb = constp.tile([HCOLS, 3], F32)
            ident_sb = constp.tile([128, 128], F32R)
            tri_sb = constp.tile([128, 128], F32R)
            nc.sync.dma_start(
                w_sb[:].rearrange("p (kb d) -> p kb d", d=3 * 128),
                w_d.rearrange("(kb p) d -> p kb d", p=128),
            )
            nc.sync.dma_start(ow_sb[:], ow_d[:])
            nc.sync.dma_start(bias_sb[:], bias_d[:])
            nc.sync.dma_start(ident_sb[:], ident_d[:])
            nc.sync.dma_start(tri_sb[:], tri_d[:])
            w3 = w_sb[:].rearrange("p (kb d) -> p kb d", d=3 * 128)

            xT = xtp.tile([128, KB * TH], F32R)               # [k, (kb, t)] half batch
            qT = qkvp.tile([128, B * T], F32R, name="qT")     # [2 heads*64, (b, t)]
            kT = qkvp.tile([128, B * T], F32R, name="kT")
            vT = qkvp.tile([128, T], F32R, name="vT")         # current batch
            attT = attp.tile([128, B * T], F32R)

            def xpose_unit(b, half, tbl):
                """Transpose one 128-row block of x into xT (8 kb blocks)."""
                tb = (TB // 2) * half + tbl

                def run():
                    xnat = xnatp.tile([128, C], F32R, name="xnat")
                    nc.sync.dma_start(xnat[:], x_d[b, tb * 128:(tb + 1) * 128, :])
                    pt = ps.tile([128, 1024], F32R, name="psx", tag="b2",
                                 bufs=CFG["b2"])
                    for kb in range(KB):
                        nc.tensor.transpose(
                            pt[:, kb * 128:(kb + 1) * 128],
                            xnat[:, kb * 128:(kb + 1) * 128],
                            ident_sb[:],
                        )
                    dst = xT[:].rearrange("p (kb t) -> p kb t", t=TH)[
                        :, :, tbl * 128:(tbl + 1) * 128]
                    src = pt[:].rearrange("p (i t) -> p i t", t=128)
                    nc.vector.tensor_copy(dst, src)
                return run

            def qkv_unit(b, half, m, tcl):
                """One 512-wide t-chunk of q/k/v^T projection."""
                tc_ = (NCH // 2) * half + tcl
                dstT = (qT, kT, vT)[m]

                def run():
                    pt = ps.tile([128, 512], F32, name="psqkv", tag="b1",
                                 bufs=CFG["b1"])
                    for kb in range(KB):
                        nc.tensor.matmul(
                            pt[:],
                            w3[:, kb, m * 128:(m + 1) * 128],
                            xT[:, kb * TH + tcl * 512: kb * TH + tcl * 512 + 512],
                            start=(kb == 0),
                            stop=(kb == KB - 1),
                        )
                    vdst = dstT[:, tc_ * 512:tc_ * 512 + 512] if m == 2 \
                        else dstT[:, b * T + tc_ * 512: b * T + tc_ * 512 + 512]
                    nc.vector.tensor_scalar_add(vdst, pt[:], bias_sb[:, m:m + 1])
                return run

            def xpose_qkv_units(b):
                units = []
                for half in range(2):
                    for tbl in range(TB // 2):
                        units.append(xpose_unit(b, half, tbl))
                    for m in range(3):
                        for tcl in range(NCH // 2):
                            units.append(qkv_unit(b, half, m, tcl))
                return units

            def vaug_units(b, vaugs):
                """Build v_aug tiles for both heads of batch b (2 units)."""
                units = []
                for h in range(HL):
                    def run(h=h):
                        va = vaugp.tile([128, TB * 65], F32R, name=f"vaug{h}",
                                        tag=f"va{h}")
                        nc.sync.dma_start(
                            va[:].rearrange("p (tb d) -> p tb d", d=65)[:, :, 64:65],
                            ones_d.rearrange("p (tb o) -> p tb o", o=1))
                        pt = ps.tile([128, 1024], F32R, name="psva", tag="b2",
                                     bufs=CFG["b2"])
                        for tb in range(TB):
                            nc.tensor.transpose(
                                pt[:, tb * 64:(tb + 1) * 64],
                                vT[h * 64:(h + 1) * 64, tb * 128:(tb + 1) * 128],
                                ident_sb[h * 64:(h + 1) * 64, h * 64:(h + 1) * 64],
                            )
                        dst = va[:].rearrange("p (tb d) -> p tb d", d=65)[:, :, 0:64]
                        src = pt[:].rearrange("p (i d) -> p i d", d=64)
                        nc.vector.tensor_copy(dst, src)
                        vaugs[h] = va
                    units.append(run)
                return units

            def attention_units(b, vaugs):
                """Unit per j-block pair (scores+exp+PV both heads); the last
                unit of each i-chunk also normalizes into attT."""
                units = []
                for ic in range(NCH):
                    i0 = 512 * ic
                    n_jb = 4 * (ic + 1)
                    pv = [None, None]

                    for g in range(n_jb // 2):
                        def run(g=g, ic=ic, i0=i0, n_jb=n_jb, pv=pv):
                            if g == 0:
                                for h in range(HL):
                                    pv[h] = ps.tile([128, 512], F32,
                                                    name=f"pspv{h}", tag="pv",
                                                    bufs=CFG["pv"])
                            sc = [ps.tile([128, 1024], F32, name=f"pssc{h}",
                                          tag="b2", bufs=CFG["b2"])
                                  for h in range(HL)]
                            for u in range(2):
                                jb = 2 * g + u
                                j0 = 128 * jb
                                for h in range(HL):
                                    nc.tensor.matmul(
                                        sc[h][:, u * 512:(u + 1) * 512],
                                        kT[h * 64:(h + 1) * 64,
                                           b * T + j0: b * T + j0 + 128],
                                        qT[h * 64:(h + 1) * 64,
                                           b * T + i0: b * T + i0 + 512],
                                        start=True, stop=True,
                                    )
                            pts = []
                            for h in range(HL):
                                pt_sb = ptp.tile([128, 1024], F32R, name=f"pt{h}",
                                                 tag=f"pt{h}", bufs=CFG["pt"])
                                nc.scalar.activation(
                                    pt_sb[:], sc[h][:], Exp,
                                    scale=float(1.0 / np.sqrt(DH)))
                                pts.append(pt_sb)
                            for u in range(2):
                                jb = 2 * g + u
                                o = 128 * jb - i0
                                lo = max(o, 0)
                                for h in range(HL):
                                    if o >= 0:
                                        seg = pts[h][:, u * 512 + o:
                                                     u * 512 + o + 128]
                                        nc.vector.tensor_tensor(
                                            seg, seg, tri_sb[:],
                                            mybir.AluOpType.mult)
                                    nc.tensor.matmul(
                                        pv[h][0:65, lo:512],
                                        vaugs[h][:, jb * 65: jb * 65 + 65],
                                        pts[h][:, u * 512 + lo:(u + 1) * 512],
                                        start=(jb == 0),
                                        stop=(jb == n_jb - 1),
                                    )
                            if 2 * g + 1 == n_jb - 1:   # chunk done: normalize
                                for h in range(HL):
                                    rrow = recipp.tile([1, 512], F32,
                                                       name="rrow", tag="rr")
                                    rbc = recipp.tile([64, 512], F32,
                                                      name="rbc", tag="rb")
                                    nc.vector.reciprocal(rrow[:], pv[h][64:65, :])
                                    nc.gpsimd.partition_broadcast(rbc[:], rrow[:])
                                    nc.vector.tensor_tensor(
                                        attT[h * 64:(h + 1) * 64,
                                             b * T + i0: b * T + i0 + 512],
                                        pv[h][0:64, :], rbc[:],
                                        mybir.AluOpType.mult)
                        units.append(run)
                return units

            def oproj_units(b, act_share=False):
                units = []
                for tb in range(TB):
                    def run(tb=tb):
                        yo = youtp.tile([128, C], F32, name="yo")
                        for ec in range(2):
                            pt = ps.tile([128, 512], F32, name="psy", tag="b1",
                                         bufs=CFG["b1"])
                            nc.tensor.matmul(
                                pt[:],
                                attT[:, b * T + tb * 128: b * T + (tb + 1) * 128],
                                ow_sb[:, ec * 512:(ec + 1) * 512],
                                start=True, stop=True,
                            )
                            dst = yo[:, ec * 512:(ec + 1) * 512]
                            if ec == 0 or not act_share:
                                nc.vector.tensor_copy(dst, pt[:])
                            else:
                                nc.scalar.activation(
                                    dst, pt[:], mybir.ActivationFunctionType.Copy)
                        nc.sync.dma_start(
                            y_d[b * T + tb * 128: b * T + (tb + 1) * 128, :], yo[:])
                    units.append(run)
                return units

            # ---- schedule ----
            va0, va1 = [None, None], [None, None]
            for u in xpose_qkv_units(0):
                u()
            for u in vaug_units(0, va0):
                u()
            _interleave(attention_units(0, va0),
                        xpose_qkv_units(1) + vaug_units(1, va1))
            if dbg:
                nc.sync.dma_start(qT_dbg[:], qT[:].bitcast(F32))
                nc.sync.dma_start(kT_dbg[:], kT[:].bitcast(F32))
            a1 = attention_units(1, va1)
            o1 = oproj_units(1, act_share=True)
            seq = []
            oi = 0
            for i, u in enumerate(a1):
                seq.append(u)
                if i in (1, 5, 11, 19):       # i-chunk of attn(1) complete
                    seq.extend(o1[oi:oi + 4])
                    oi += 4
            _interleave(seq, oproj_units(0))
            if dbg:
                nc.sync.dma_start(attT_dbg[:], attT[:].bitcast(F32))

    nc.compile()
    return nc


def _prep_inputs(x, qkv_w, qkv_b, o_w):
    """Per-core input maps (head sharding)."""
    ident = np.eye(128, dtype=np.float32)
    tri = np.triu(np.ones((128, 128), dtype=np.float32))
    x = np.ascontiguousarray(np.asarray(x, dtype=np.float32))
    qkv_w = np.asarray(qkv_w, dtype=np.float32)
    qkv_b = np.asarray(qkv_b, dtype=np.float32)
    o_w = np.asarray(o_w, dtype=np.float32)
    in_maps = []
    for c in range(NCORES):
        lo = c * HCOLS
        w_c = np.concatenate(
            [qkv_w[:, lo:lo + HCOLS],
             qkv_w[:, C + lo:C + lo + HCOLS],
             qkv_w[:, 2 * C + lo:2 * C + lo + HCOLS]], axis=1)
        b_c = np.stack(
            [qkv_b[lo:lo + HCOLS],
             qkv_b[C + lo:C + lo + HCOLS],
             qkv_b[2 * C + lo:2 * C + lo + HCOLS]], axis=1)
        ow_c = o_w[lo:lo + HCOLS, :]
        in_maps.append({
            "x": x,
            "w": np.ascontiguousarray(w_c),
            "bqkv": np.ascontiguousarray(b_c),
            "ow": np.ascontiguousarray(ow_c),
            "ident": ident,
            "tri": tri,
            "ones": np.ones((128, TB), dtype=np.float32),
        })
    return in_maps


def kernel(x, qkv_w, qkv_b, o_w, o_b):
    global _nc_cache
    from concourse import bass_utils
    if _nc_cache is None:
        _nc_cache = build_bass()
    nc = _nc_cache
    in_maps = _prep_inputs(x, qkv_w, qkv_b, o_w)
    res = bass_utils.run_bass_kernel_spmd(nc, in_maps, core_ids=list(range(NCORES)))
    y = np.zeros((B * T, C), dtype=np.float64)
    for c in range(NCORES):
        y += res.results[c]["y"].astype(np.float64)
    y = (y + np.asarray(o_b, dtype=np.float64)[None, :]).astype(np.float32)
    return y.reshape(B, T, C)
